# revision 6
# baseline (speedup 1.0000x reference)
"""Conv2d (32,128,64,64) x (256,128,3,3) stride 1 pad 1 -> (32,256,64,64), f32.

Strategy: data-parallel over batch across 8 NeuronCores (4 images/core).
Per core, conv is computed as 9 PSUM-accumulated matmuls (one per kernel tap):
  out[o, y, x] += W[o, i, kh, kw] * xpad[i, y+kh, x+kw]
with contraction over i (=128, the partition dim), lhsT = W transposed to
[i, o] per tap (transposed on-chip via TensorE), and rhs read from a
zero-padded [128, 66, 66] SBUF image with a strided 2-D free access pattern.
Each matmul covers 8 output rows (N = 512) into one PSUM bank; matmul operands
are bitcast to float32r (full fp32 bits, 1 cycle/row on the PE vs 4 for
plain fp32). Bias is fused into the PSUM->SBUF drain on the vector engine.
"""

import numpy as np

B, CIN, H, W = 32, 128, 64, 64
COUT, KH, KW = 256, 3, 3
N_CORES = 8
B_LOC = B // N_CORES            # images per core
HP, WP = H + 2, W + 2           # padded image
ROWS = 8                        # output rows per matmul
NBLK = H // ROWS                # spatial blocks per image
NOC = COUT // 128               # output-channel chunks
NK = KH * KW

_CACHE: dict = {}


def _build():
    import concourse.bacc as bacc
    import concourse.mybir as mybir
    import concourse.tile as tile
    from concourse.masks import make_identity

    f32 = mybir.dt.float32
    f32r = mybir.dt.float32r

    nc = bacc.Bacc(
        "TRN2",
        target_bir_lowering=False,
        debug=False,
        enable_asserts=False,
        num_devices=N_CORES,
    )
    x_d = nc.dram_tensor("input", (B_LOC, CIN, H, W), f32, kind="ExternalInput").ap()
    w_d = nc.dram_tensor("weights", (COUT, CIN, KH, KW), f32, kind="ExternalInput").ap()
    b_d = nc.dram_tensor("biases", (COUT,), f32, kind="ExternalInput").ap()
    y_d = nc.dram_tensor("out", (B_LOC, COUT, H, W), f32, kind="ExternalOutput").ap()

    with tile.TileContext(nc) as tc:
        with (
            tc.tile_pool(name="const", bufs=1) as const_pool,
            tc.tile_pool(name="wstage", bufs=2) as wstage_pool,
            tc.tile_pool(name="wT", bufs=1) as wT_pool,
            tc.tile_pool(name="xpad", bufs=3) as x_pool,
            tc.tile_pool(name="outsb", bufs=2) as out_pool,
            tc.tile_pool(name="psum", bufs=6, space="PSUM") as psum_pool,
            tc.tile_pool(name="psum_t", bufs=2, space="PSUM") as psumt_pool,
        ):
            ident = const_pool.tile([128, 128], f32)
            make_identity(nc, ident[:, :])

            # biases (256,) -> [o', oc] so bias_t[:, oc] is per-partition
            bias_t = const_pool.tile([128, NOC], f32)
            nc.sync.dma_start(bias_t[:, :], b_d.rearrange("(a p) -> p a", p=128))

            # Weights: load OIHW contiguous, then 18x PE-transpose of
            # [128 o, 128 i] (free stride 9) -> wT[i, tap*NOC+oc, o].
            # The DVE copy out of PSUM rounds to fp32r for the matmuls.
            wT = wT_pool.tile([128, NK * NOC, 128], f32r)
            for oc in range(NOC):
                wst = wstage_pool.tile([128, CIN, KH, KW], f32)
                nc.sync.dma_start(wst[:, :, :, :], w_d[oc * 128:(oc + 1) * 128])
                for kk in range(NK):
                    kh, kw = kk // KW, kk % KW
                    pt = psumt_pool.tile([128, 128], f32)
                    nc.tensor.transpose(pt[:, :], wst[:, :, kh, kw], ident[:, :])
                    nc.vector.tensor_copy(wT[:, kk * NOC + oc, :], pt[:, :])

            for b in range(B_LOC):
                xp = x_pool.tile([128, HP, WP], f32r)
                # zero the halo ring; interior is fully overwritten by the DMA
                nc.vector.memset(xp[:, 0, :].bitcast(f32), 0.0)
                nc.vector.memset(xp[:, HP - 1, :].bitcast(f32), 0.0)
                nc.vector.memset(xp[:, 1:H + 1, 0].bitcast(f32), 0.0)
                nc.vector.memset(xp[:, 1:H + 1, WP - 1].bitcast(f32), 0.0)
                # raw-byte HWDGE DMA; the PE rounds fp32r operands on read
                nc.sync.dma_start(xp[:, 1:H + 1, 1:W + 1], x_d[b].bitcast(f32r))

                for oc in range(NOC):
                    # whole [128, 64, 64] output half staged in SBUF -> one 2 MB DMA
                    ot = out_pool.tile([128, H * W], f32)
                    for s in range(NBLK):
                        ps = psum_pool.tile([128, ROWS * W], f32)
                        for kk in range(NK):
                            kh, kw = kk // KW, kk % KW
                            rhs = xp[:, s * ROWS + kh: s * ROWS + kh + ROWS, kw: kw + W]
                            nc.tensor.matmul(
                                ps[:, :],
                                wT[:, kk * NOC + oc, :],
                                rhs,
                                start=(kk == 0),
                                stop=(kk == NK - 1),
                            )
                        nc.vector.tensor_scalar_add(
                            ot[:, s * ROWS * W:(s + 1) * ROWS * W],
                            ps[:, :],
                            bias_t[:, oc:oc + 1],
                        )
                    nc.sync.dma_start(
                        y_d[b, oc * 128:(oc + 1) * 128, :, :], ot[:, :]
                    )

    nc.compile()
    return nc


def get_nc():
    if "nc" not in _CACHE:
        _CACHE["nc"] = _build()
    return _CACHE["nc"]


def kernel(input, weights, biases):
    from concourse import bass_utils

    nc = get_nc()
    input = np.ascontiguousarray(input, dtype=np.float32)
    shards = input.reshape(N_CORES, B_LOC, CIN, H, W)
    in_maps = [
        {
            "input": shards[c],
            "weights": np.ascontiguousarray(weights, dtype=np.float32),
            "biases": np.ascontiguousarray(biases, dtype=np.float32),
        }
        for c in range(N_CORES)
    ]
    res = bass_utils.run_bass_kernel_spmd(nc, in_maps, core_ids=list(range(N_CORES)))
    return np.concatenate([res.results[c]["out"] for c in range(N_CORES)], axis=0)


# revision 8
# speedup vs baseline: 1.2283x; 1.2283x over previous
"""Conv2d (32,128,64,64) x (256,128,3,3) stride 1 pad 1 -> (32,256,64,64), f32.

Strategy: data-parallel over batch across 8 NeuronCores (4 images/core).
Per core, conv is computed as 9 PSUM-accumulated matmuls (one per kernel tap):
  out[o, y, x] += W[o, i, kh, kw] * xpad[i, y+kh, x+kw]
with contraction over i (=128, the partition dim), lhsT = W transposed to
[i, o] per tap (transposed on-chip via TensorE), and rhs read from a
zero-padded [128, 66, 66] SBUF image with a strided 2-D free access pattern.
Each matmul covers 8 output rows (N = 512) into one PSUM bank; matmul operands
are bitcast to float32r (full fp32 bits, 1 cycle/row on the PE vs 4 for
plain fp32). Bias is fused into the PSUM->SBUF drain on the vector engine.
"""

import numpy as np

B, CIN, H, W = 32, 128, 64, 64
COUT, KH, KW = 256, 3, 3
N_CORES = 8
B_LOC = B // N_CORES            # images per core
HP, WP = H + 2, W + 2           # padded image
ROWS = 8                        # output rows per matmul
NBLK = H // ROWS                # spatial blocks per image
NOC = COUT // 128               # output-channel chunks
NK = KH * KW

_CACHE: dict = {}


def _build():
    import concourse.bacc as bacc
    import concourse.mybir as mybir
    import concourse.tile as tile
    from concourse.masks import make_identity

    f32 = mybir.dt.float32
    f32r = mybir.dt.float32r

    nc = bacc.Bacc(
        "TRN2",
        target_bir_lowering=False,
        debug=False,
        enable_asserts=False,
        num_devices=N_CORES,
    )
    x_d = nc.dram_tensor("input", (B_LOC, CIN, H, W), f32, kind="ExternalInput").ap()
    w_d = nc.dram_tensor("weights", (COUT, CIN, KH, KW), f32, kind="ExternalInput").ap()
    b_d = nc.dram_tensor("biases", (COUT,), f32, kind="ExternalInput").ap()
    y_d = nc.dram_tensor("out", (B_LOC, COUT, H, W), f32, kind="ExternalOutput").ap()

    with tile.TileContext(nc) as tc:
        with (
            tc.tile_pool(name="const", bufs=1) as const_pool,
            tc.tile_pool(name="wstage", bufs=2) as wstage_pool,
            tc.tile_pool(name="wT", bufs=1) as wT_pool,
            tc.tile_pool(name="xpad", bufs=3) as x_pool,
            tc.tile_pool(name="outsb", bufs=2) as out_pool,
            tc.tile_pool(name="psum", bufs=6, space="PSUM") as psum_pool,
            tc.tile_pool(name="psum_t", bufs=2, space="PSUM") as psumt_pool,
        ):
            ident = const_pool.tile([128, 128], f32)
            make_identity(nc, ident[:, :])

            # biases (256,) -> [o', oc] so bias_t[:, oc] is per-partition
            bias_t = const_pool.tile([128, NOC], f32)
            nc.sync.dma_start(bias_t[:, :], b_d.rearrange("(a p) -> p a", p=128))

            # Weights: load OIHW contiguous, then 18x PE-transpose of
            # [128 o, 128 i] (free stride 9) -> wT[i, tap*NOC+oc, o].
            # The DVE copy out of PSUM rounds to fp32r for the matmuls.
            wT = wT_pool.tile([128, NK * NOC, 128], f32r)
            for oc in range(NOC):
                wst = wstage_pool.tile([128, CIN, KH, KW], f32)
                nc.sync.dma_start(wst[:, :, :, :], w_d[oc * 128:(oc + 1) * 128])
                for kk in range(NK):
                    kh, kw = kk // KW, kk % KW
                    pt = psumt_pool.tile([128, 128], f32)
                    nc.tensor.transpose(pt[:, :], wst[:, :, kh, kw], ident[:, :])
                    nc.vector.tensor_copy(wT[:, kk * NOC + oc, :], pt[:, :])

            for b in range(B_LOC):
                xp = x_pool.tile([128, HP, WP], f32r)
                # zero the halo ring; interior is fully overwritten by the DMA
                nc.vector.memset(xp[:, 0, :].bitcast(f32), 0.0)
                nc.vector.memset(xp[:, HP - 1, :].bitcast(f32), 0.0)
                nc.vector.memset(xp[:, 1:H + 1, 0].bitcast(f32), 0.0)
                nc.vector.memset(xp[:, 1:H + 1, WP - 1].bitcast(f32), 0.0)
                # raw-byte HWDGE DMA; the PE rounds fp32r operands on read.
                # Chunked by row-groups so the first matmuls start early.
                for r0 in range(0, H, 16):
                    nc.sync.dma_start(
                        xp[:, r0 + 1:r0 + 17, 1:W + 1],
                        x_d[b, :, r0:r0 + 16, :].bitcast(f32r),
                    )

                for oc in range(NOC):
                    # whole [128, 64, 64] output half staged in SBUF -> one 2 MB DMA
                    ot = out_pool.tile([128, H * W], f32)
                    for s in range(NBLK):
                        ps = psum_pool.tile([128, ROWS * W], f32)
                        for kk in range(NK):
                            kh, kw = kk // KW, kk % KW
                            rhs = xp[:, s * ROWS + kh: s * ROWS + kh + ROWS, kw: kw + W]
                            nc.tensor.matmul(
                                ps[:, :],
                                wT[:, kk * NOC + oc, :],
                                rhs,
                                start=(kk == 0),
                                stop=(kk == NK - 1),
                            )
                        nc.vector.tensor_scalar_add(
                            ot[:, s * ROWS * W:(s + 1) * ROWS * W],
                            ps[:, :],
                            bias_t[:, oc:oc + 1],
                        )
                        if s % 2 == 1:
                            # flush two drained blocks (1 MB, contiguous in DRAM)
                            nc.sync.dma_start(
                                y_d[b, oc * 128:(oc + 1) * 128, (s - 1) * ROWS:(s + 1) * ROWS, :],
                                ot[:, (s - 1) * ROWS * W:(s + 1) * ROWS * W],
                            )

    nc.compile()
    return nc


def get_nc():
    if "nc" not in _CACHE:
        _CACHE["nc"] = _build()
    return _CACHE["nc"]


def kernel(input, weights, biases):
    from concourse import bass_utils

    nc = get_nc()
    input = np.ascontiguousarray(input, dtype=np.float32)
    shards = input.reshape(N_CORES, B_LOC, CIN, H, W)
    in_maps = [
        {
            "input": shards[c],
            "weights": np.ascontiguousarray(weights, dtype=np.float32),
            "biases": np.ascontiguousarray(biases, dtype=np.float32),
        }
        for c in range(N_CORES)
    ]
    res = bass_utils.run_bass_kernel_spmd(nc, in_maps, core_ids=list(range(N_CORES)))
    return np.concatenate([res.results[c]["out"] for c in range(N_CORES)], axis=0)
